# revision 4
# baseline (speedup 1.0000x reference)
"""Trainium2 Bass SPMD kernel: DeepPoly ReLU layer relaxation (N=8192).

Outputs (matching reference): x_out, lower_ret, upper_ret,
lower_weights (NxN diag), upper_weights (NxN diag), lower_bias, upper_bias.

Sharding: neuron dim N split across 8 cores (1024 each). Each core writes its
1024-row slab of both NxN weight matrices. The slab rows are built on-chip as
[128, 8192] tiles that are zero except the diagonal element, placed with a
fused (iota == idx) * val tensor_scalar op where idx comes from a per-core
input tensor -- so the SPMD program is identical on all cores.
"""

import sys

import numpy as np

N = 8192
NCORES = 8
SHARD = N // NCORES  # 1024 neurons per core
TPC = SHARD // 128   # 8 row-tiles of 128 rows per core

_CACHE = {}
TRACE = False
TRACE_KWARGS = {}
LAST_RESULT = None


def _import_concourse():
    try:
        import concourse.bass  # noqa: F401
    except ImportError:
        sys.path.insert(0, "/opt/trn_rl_repo")


def _build():
    _import_concourse()
    import concourse.bacc as bacc
    import concourse.tile as tile
    from concourse import mybir

    f32 = mybir.dt.float32
    op = mybir.AluOpType

    # Bacc (not raw Bass): its compile() splits multi-sem waits into
    # event-semaphore chains -- TRN2 allows at most 1 wait per instruction.
    nc = bacc.Bacc()

    x_in = nc.dram_tensor("x_s", [128, TPC], f32, kind="ExternalInput")
    l_in = nc.dram_tensor("lower_s", [128, TPC], f32, kind="ExternalInput")
    u_in = nc.dram_tensor("upper_s", [128, TPC], f32, kind="ExternalInput")
    i_in = nc.dram_tensor("idx_s", [128, TPC], f32, kind="ExternalInput")

    uw_out = nc.dram_tensor("uw", [SHARD, N], f32, kind="ExternalOutput")
    lw_out = nc.dram_tensor("lw", [SHARD, N], f32, kind="ExternalOutput")
    sm_out = nc.dram_tensor("small", [128, 5 * TPC], f32, kind="ExternalOutput")

    T = TPC
    with tile.TileContext(nc) as tc:
        with (
            tc.tile_pool(name="singles", bufs=1) as singles,
            tc.tile_pool(name="big", bufs=4) as bigpool,
        ):
            X = singles.tile([128, T], f32)
            L = singles.tile([128, T], f32)
            U = singles.tile([128, T], f32)
            IDX = singles.tile([128, T], f32)
            nc.sync.dma_start(out=X[:], in_=x_in[:, :])
            nc.sync.dma_start(out=L[:], in_=l_in[:, :])
            nc.sync.dma_start(out=U[:], in_=u_in[:, :])
            nc.sync.dma_start(out=IDX[:], in_=i_in[:, :])

            # column index ramp 0..N-1, identical in every partition
            IOTA = singles.tile([128, N], f32)
            nc.gpsimd.iota(
                IOTA[:],
                pattern=[[1, N]],
                base=0,
                channel_multiplier=0,
                allow_small_or_imprecise_dtypes=True,
            )

            # per-neuron branch math on [128, TPC] tiles
            notneg = singles.tile([128, T], f32)  # u > 0   (== lw_diag)
            nc.vector.tensor_scalar(notneg[:], U[:], 0.0, None, op.is_gt)
            lneg = singles.tile([128, T], f32)  # l < 0
            nc.vector.tensor_scalar(lneg[:], L[:], 0.0, None, op.is_lt)
            cross = singles.tile([128, T], f32)  # crossing branch
            nc.vector.tensor_mul(cross[:], notneg[:], lneg[:])
            d = singles.tile([128, T], f32)
            nc.vector.tensor_sub(d[:], U[:], L[:])
            # clamp away from 0 so recip stays finite; exact where cross=1
            # (there d = u-l > 0), and the clamped lanes are masked by cross
            dsafe = singles.tile([128, T], f32)
            nc.vector.tensor_scalar_max(dsafe[:], d[:], 1e-30)
            r = singles.tile([128, T], f32)
            nc.vector.reciprocal(r[:], dsafe[:])
            slope = singles.tile([128, T], f32)
            nc.vector.tensor_mul(slope[:], U[:], r[:])
            slopec = singles.tile([128, T], f32)  # cross * slope
            nc.vector.tensor_mul(slopec[:], cross[:], slope[:])
            pos = singles.tile([128, T], f32)  # l >= 0
            nc.vector.tensor_scalar(pos[:], L[:], 0.0, None, op.is_ge)
            tmp = singles.tile([128, T], f32)
            nc.vector.tensor_add(tmp[:], pos[:], slopec[:])
            uwd = singles.tile([128, T], f32)  # uw_diag
            nc.vector.tensor_mul(uwd[:], tmp[:], notneg[:])

            # packed small outputs: x_out | lower_ret | upper_ret | upper_bias | lower_bias
            sm = singles.tile([128, 5 * T], f32)
            nc.vector.tensor_relu(sm[:, 0:T], X[:])
            nc.vector.tensor_mul(sm[:, T : 2 * T], L[:], notneg[:])
            nc.vector.tensor_mul(sm[:, 2 * T : 3 * T], U[:], uwd[:])
            nc.vector.tensor_mul(sm[:, 3 * T : 4 * T], slopec[:], L[:])
            nc.vector.memset(sm[:, 4 * T : 5 * T], 0.0)
            nc.sync.dma_start(out=sm_out[:, :], in_=sm[:])

            # weight slabs: one [128, N] source tile per (matrix, row-tile),
            # zero except the diagonal element per row
            for t in range(T):
                for m, (wout, val) in enumerate(((uw_out, uwd), (lw_out, notneg))):
                    big = bigpool.tile([128, N], f32)
                    nc.vector.tensor_scalar(
                        big[:],
                        IOTA[:],
                        IDX[:, t : t + 1],
                        val[:, t : t + 1],
                        op.is_equal,
                        op.mult,
                    )
                    eng = nc.sync if m == 0 else nc.scalar
                    eng.dma_start(
                        out=wout[t * 128 : (t + 1) * 128, :], in_=big[:]
                    )
    nc.compile()
    return nc


def _shard2d(v):
    # (1024,) -> [128, TPC] with (p, t) holding element t*128+p
    return np.ascontiguousarray(v.reshape(TPC, 128).T.astype(np.float32))


def kernel(x, lower, upper, input_shape=None, **_unused):
    global LAST_RESULT
    _import_concourse()
    from concourse import bass_utils

    x = np.asarray(x, dtype=np.float32).reshape(N)
    lower = np.asarray(lower, dtype=np.float32).reshape(N)
    upper = np.asarray(upper, dtype=np.float32).reshape(N)

    if "nc" not in _CACHE:
        _CACHE["nc"] = _build()
    nc = _CACHE["nc"]

    in_maps = []
    for c in range(NCORES):
        sl = slice(c * SHARD, (c + 1) * SHARD)
        in_maps.append(
            {
                "x_s": _shard2d(x[sl]),
                "lower_s": _shard2d(lower[sl]),
                "upper_s": _shard2d(upper[sl]),
                "idx_s": _shard2d(
                    np.arange(c * SHARD, (c + 1) * SHARD, dtype=np.float32)
                ),
            }
        )

    res = bass_utils.run_bass_kernel_spmd(
        nc,
        in_maps,
        core_ids=list(range(NCORES)),
        trace=TRACE,
        **TRACE_KWARGS,
    )
    LAST_RESULT = res
    cores = res.results

    upper_weights = np.concatenate([r["uw"] for r in cores], axis=0)
    lower_weights = np.concatenate([r["lw"] for r in cores], axis=0)

    def unpack(col):
        parts = [
            cores[c]["small"][:, col * TPC : (col + 1) * TPC].T.reshape(-1)
            for c in range(NCORES)
        ]
        return np.concatenate(parts).reshape(1, N)

    x_out = unpack(0)
    lower_ret = unpack(1)
    upper_ret = unpack(2)
    upper_bias = unpack(3)
    lower_bias = unpack(4)

    return (
        x_out,
        lower_ret,
        upper_ret,
        lower_weights,
        upper_weights,
        lower_bias,
        upper_bias,
    )
